# revision 26
# baseline (speedup 1.0000x reference)
"""Trainium2 Bass kernel: image -> additive-sinusoid audio encoding.

Math (per batch image b):
  gray = 255 * (w . rgb);  rev = flip(gray, rows);  avg = mean(gray)
  px   = clip(3*rev - 2*avg, 0, 255)
  A    = where(px==0, 0, exp(ln10 * (px/160 - 1.5)))            # [M=64 rows, N=64 cols]
  y[t] = sum_m A[m, col(t)] * sin(W[m]*t*dt + PHI0[m]),  col(t) = min(t//361, 63)
  audio= clip(0.5 + 2048*y, -32768, 32767)                       # [ns=23152]

Kernel strategy: t = n*361 + r  =>  angle = theta[i,n] + beta[i,r] (row flip folded
into the host tables), so  sinmat = sin(theta)cos(beta) + cos(theta)sin(beta).
Data-parallel over batch: 8 images per NeuronCore.

SBUF layout: partition p = (h, i) with h in {sin-half, cos-half} and i the image
row; the host ships 3*gray255 in fp16, duplicated onto both halves (so no
on-device PE duplication is needed), free = (b local image, n column), plus 4
per-row-sum columns per image-half (host-folded, same class as the grayscale
weight fold).  Per image pair g:
  y_g[(b2,n), r] = sum_{(h,i)} PQ[(h,i),(b2,n)] * CS[(h,i),r]      (N=361)
Pipeline per half: one ones-matmul turns the row sums into 2*avg broadcast on
every partition (cross-partition sum + broadcast in one shot; the ones matrix
is memset on device so the table DMA stays off the critical path); px = G3 -
2avg on DVE (kept UNCLIPPED: exp(px>255) is finite in f16); E = exp on ACT; A =
(px>0)*E and PQ = min(A, A255)*TT as two DVE stt ops - the 255-clip rides the
spare ALU slot of the PQ op.  Drains of the four PSUM results are split
DVE/ACT at col 225 and stream out as two HWDGE DMAs (g0g1 on the SP ring, g2g3
on the ACT ring).  The 48 col-63 tail samples come from one tiny [8,48] matmul
whose result is parked in audio_b's spare columns.  Input arrives as two G
half-DMAs (first half unblocks compute earlier) followed by the table DMA on
the same SP ring.
Host side: output comes back as fp16 y+0.5 and is clipped/cast during the
unshard gather.
"""

import os

import numpy as np

# ---- problem constants (from the nn.Module definition; input-independent) ----
M = 64
N = 64
FL, FH, FS, T = 80.0, 7600.0, 22050, 1.05
NS = 2 * int(0.5 * FS * T)  # 23152
NUM = NS // N  # 361
RMAX = NS - (N - 1) * NUM  # 409 (last column's sample count)
RMAIN = NUM  # 361 samples per column in the main matmuls
RT = RMAX - NUM  # 48-sample tail of column 63
DT = float(np.float32(1.0 / FS))  # reference rounds dt to f32 (jnp weak typing)
TWO_PI = 2.0 * np.pi
B = 64
N_CORES = 8
B_LOC = B // N_CORES  # 8 images per core
SCALE_SSM = (0.5 / np.sqrt(M)) * 32768.0  # 2048
LN10 = float(np.log(10.0))
EXP_A = LN10 / 160.0
EXP_B = -1.5 * LN10
W0, W1, W2 = 0.2989, 0.5870, 0.1140
ONES_VAL = 1.0 / (3.0 * 4096.0)  # 128 dup'd partitions of 3*gray -> 2*avg

# single table: [TT 512 | CS 409 | pad 1]
C_TT, C_CS = 0, 512
TABW = 512 + RMAX + 1  # 922, keeps row stride 4B-aligned


def _make_tables():
    # LCG phase bank (faithful port, ir starts at 0)
    ia, ic, im = 9301, 49297, 233280
    ir = 0
    phi = []
    for _ in range(M):
        ir = (ir * ia + ic) % im
        phi.append(TWO_PI * ir / im)
    phi32 = np.array(phi, np.float64).astype(np.float32)
    w32 = (TWO_PI * FL * (FH / FL) ** (np.arange(M) / (M - 1))).astype(np.float32)

    # fold the row flip (tf.reverse on axis 1) into the tables: row i uses W[63-i]
    wf = w32[::-1].astype(np.float64)
    phif = phi32[::-1].astype(np.float64)

    n_idx = np.arange(N, dtype=np.float64)
    theta = wf[:, None] * (n_idx[None, :] * NUM * DT) + phif[:, None]  # [64, 64]
    stct = np.concatenate([np.sin(theta), np.cos(theta)], axis=0)  # [128, 64]
    tt = np.tile(stct[:, None, :], (1, B_LOC, 1)).reshape(128, 512)

    r_idx = np.arange(RMAX, dtype=np.float64)
    beta = wf[:, None] * (r_idx[None, :] * DT)  # [64, 409]
    cs = np.concatenate(
        [SCALE_SSM * np.cos(beta), SCALE_SSM * np.sin(beta)], axis=0
    )  # [128, 409]

    pad = np.zeros((128, 1))
    tabs = np.concatenate([tt, cs, pad], axis=1).astype(np.float16)
    assert tabs.shape == (128, TABW), tabs.shape
    return {"tabs": np.ascontiguousarray(tabs)}


_TABLES = None


def tables():
    global _TABLES
    if _TABLES is None:
        _TABLES = _make_tables()
    return _TABLES


def build_nc():
    import concourse.bacc as bacc
    import concourse.bass as bass
    import concourse.mybir as mybir
    import concourse.tile as tile

    f32 = mybir.dt.float32
    f16 = mybir.dt.float16
    Alu = mybir.AluOpType
    Act = mybir.ActivationFunctionType

    nc = bacc.Bacc(
        "TRN2",
        target_bir_lowering=False,
        debug=False,
        num_devices=N_CORES,
        enable_asserts=False,
    )

    # per half: 256 pixel cols + 4 row-sum cols (host-folded, like the
    # grayscale weights) -> [img0-3 | rs0-3 | img4-7 | rs4-7]
    g16_d = nc.dram_tensor("g16", [128, 520], f16, kind="ExternalInput")
    tabs_d = nc.dram_tensor("tabs", [128, TABW], f16, kind="ExternalInput")
    audio_a_d = nc.dram_tensor("audio_a", [128, 2, RMAIN], f16, kind="ExternalOutput")
    audio_b_d = nc.dram_tensor("audio_b", [128, 2 * RMAIN + RT], f16, kind="ExternalOutput")

    with tile.TileContext(nc) as tc:
        with (
            tc.tile_pool(name="work", bufs=1) as work,
            tc.tile_pool(name="psum", bufs=1, space=bass.MemorySpace.PSUM) as psum,
        ):
            # device-built constants: ones matrix for the avg matmul (Pool),
            # Exp activation bias column (scalar engine requires an AP bias)
            ones = work.tile([128, 128], f16)
            nc.gpsimd.memset(ones, ONES_VAL)
            expb = work.tile([128, 1], f32)
            nc.vector.memset(expb, float(EXP_B))

            # ---- input DMAs on separate HWDGE rings ----
            G = work.tile([128, 520], f16)
            TB = work.tile([128, TABW], f16)
            nc.sync.dma_start(out=G[:, 0:260], in_=g16_d[:, 0:260])
            nc.sync.dma_start(out=G[:, 260:520], in_=g16_d[:, 260:520])
            nc.sync.dma_start(out=TB, in_=tabs_d[:])
            TT = TB[:, C_TT : C_TT + 512]
            CS = TB[:, C_CS : C_CS + RMAX]

            # ---- mean path, split per image-half: the host ships per-row
            # sums; one ones-matmul per half reduces them across partitions
            # and broadcasts 2*avg everywhere (half 0 unblocks px0 early) ----
            H = B_LOC // 2  # 4 images per half
            Gh = [G[:, 0:260], G[:, 260:520]]
            cs2 = []
            for s in range(2):
                ct = psum.tile([128, H], f32, name=f"cs{s}")
                nc.tensor.matmul(ct, ones, Gh[s][:, 256:260], start=True, stop=True)
                cs2.append(ct)

            # ---- px = G3 - 2*avg -> min(255) -> exp; mask -> PQ ----
            px = work.tile([128, B_LOC, 64], f16)
            E = work.tile([128, 512], f16)
            Ah = [work.tile([128, 256], f16, name=f"A{s}") for s in range(2)]
            PQ = work.tile([128, 512], f16)
            # px is left unclipped: exp(px>255) stays finite in f16 and the
            # 255-cap is applied as min(A, A_MAX) inside the PQ stt below
            for s in range(2):
                sl = slice(s * H, (s + 1) * H)
                nc.vector.tensor_sub(
                    out=px[:, sl],
                    in0=Gh[s][:, 0:256].rearrange("p (b n) -> p b n", b=H),
                    in1=cs2[s].broadcast_to([128, H, 64]),
                )
            for s in range(2):
                sl = slice(s * H, (s + 1) * H)
                fl = slice(s * 256, (s + 1) * 256)
                nc.scalar.activation(
                    out=E[:, fl], in_=px[:, sl].rearrange("p a b -> p (a b)"),
                    func=Act.Exp, bias=expb, scale=float(EXP_A),
                )
            for s in range(2):
                sl = slice(s * H, (s + 1) * H)
                fl = slice(s * 256, (s + 1) * 256)
                nc.vector.scalar_tensor_tensor(
                    out=Ah[s], in0=px[:, sl].rearrange("p a b -> p (a b)"),
                    scalar=0.0, in1=E[:, fl], op0=Alu.is_gt, op1=Alu.mult,
                )
                nc.vector.scalar_tensor_tensor(
                    out=PQ[:, fl], in0=Ah[s], scalar=float(10.0 ** 0.09375),
                    in1=TT[:, fl], op0=Alu.min, op1=Alu.mult,
                )

            # ---- one K=128 N=361 matmul per image pair + balanced split
            # drains; col-63 tail samples come from one tiny extra matmul ----
            Ua = work.tile([128, 2, RMAIN], f16)
            # Ub: [g2 | g3 | 48 tail cols on partitions 0:8]
            Ub = work.tile([128, 2 * RMAIN + RT], f16)
            HCUT = 260
            for g in range(4):
                yt = psum.tile([128, RMAIN], f32, name=f"y{g}")
                nc.tensor.matmul(
                    yt, PQ[:, 128 * g : 128 * (g + 1)], CS[:, 0:RMAIN],
                    start=True, stop=True,
                )
                if g < 2:
                    U = Ua[:, g]
                else:
                    U = Ub[:, (g - 2) * RMAIN : (g - 1) * RMAIN]
                nc.vector.tensor_scalar(
                    out=U[:, 0:HCUT], in0=yt[:, 0:HCUT],
                    scalar1=0.5, scalar2=0.0, op0=Alu.add, op1=Alu.bypass,
                )
                nc.scalar.activation(
                    out=U[:, HCUT:RMAIN], in_=yt[:, HCUT:RMAIN],
                    func=Act.Copy, bias=0.5, scale=1.0,
                )
                if g == 1:
                    nc.sync.dma_start(out=audio_a_d[:], in_=Ua)
            # tail: y[b, r] for col 63, r in [361,409), all 8 images at once;
            # parked in Ub's spare columns so it rides the audio_b DMA
            ytt = psum.tile([8, RT], f32, name="ytail")
            nc.tensor.matmul(
                ytt, PQ.rearrange("p (b n) -> p b n", b=B_LOC)[:, :, 63],
                CS[:, RMAIN:RMAX], start=True, stop=True,
            )
            nc.vector.tensor_scalar(
                out=Ub[0:8, 2 * RMAIN :], in0=ytt, scalar1=0.5, scalar2=0.0,
                op0=Alu.add, op1=Alu.bypass,
            )
            nc.scalar.dma_start(out=audio_b_d[:], in_=Ub)

    nc.compile()
    return nc


_NC = None


def _get_nc():
    global _NC
    if _NC is None:
        _NC = build_nc()
    return _NC


LAST_RESULTS = None


def kernel(x: np.ndarray) -> np.ndarray:
    from concourse.bass_utils import run_bass_kernel_spmd

    x = np.asarray(x, dtype=np.float32)
    assert x.shape == (B, 64, 64, 3), x.shape

    # shard + permute to the SBUF layout [p=(h,i), (b, n)], fp16 grayscale,
    # duplicated across the two partition halves (sin/cos banks)
    # the contrast-stretch 3x is folded into the host grayscale (px = G3 - 2*avg)
    gray = 765.0 * (x[..., 0] * W0 + x[..., 1] * W1 + x[..., 2] * W2)  # [B,64,64]
    gc = gray.reshape(N_CORES, B_LOC, 64, 64)  # [core, b, i, n]
    gt = gc.transpose(0, 2, 1, 3)  # [core, i, b, n]
    g16px = gt.astype(np.float16)
    rows = g16px.astype(np.float32).sum(axis=3).astype(np.float16)  # [core, i, b]
    halves = []
    for s in range(2):
        halves.append(g16px[:, :, 4 * s : 4 * s + 4].reshape(N_CORES, 64, 256))
        halves.append(rows[:, :, 4 * s : 4 * s + 4])
    g1 = np.concatenate(halves, axis=2)  # [core, 64, 520]
    g16 = np.tile(g1, (1, 2, 1)).astype(np.float16)  # [core, 128, 520]

    nc = _get_nc()
    tbl = tables()
    in_maps = []
    for c in range(N_CORES):
        m = {"g16": np.ascontiguousarray(g16[c])}
        m.update(tbl)
        in_maps.append(m)

    trace = os.environ.get("BASS_KERNEL_TRACE", "0") == "1"
    res = run_bass_kernel_spmd(
        nc, in_maps, core_ids=list(range(N_CORES)), trace=trace
    )
    global LAST_RESULTS
    LAST_RESULTS = res

    outs = np.empty((B, NS), np.float32)
    for c, r in enumerate(res.results):
        aa = r["audio_a"].astype(np.float32).reshape(2, 64, 2, RMAIN)
        bb_raw = r["audio_b"].astype(np.float32)  # [128, 2*RMAIN+RT]
        bb = bb_raw[:, : 2 * RMAIN].reshape(2, 64, 2, RMAIN)
        tt_ = bb_raw[0:8, 2 * RMAIN :]  # [8, RT] tail rows
        for b_loc in range(B_LOC):
            g, b2 = b_loc // 2, b_loc % 2
            img = aa[b2, :, g] if g < 2 else bb[b2, :, g - 2]  # [64 cols, RMAIN]
            row = c * B_LOC + b_loc
            outs[row, : N * NUM] = img.reshape(N * NUM)
            outs[row, N * NUM :] = tt_[b_loc]
    np.clip(outs, -32768.0, 32767.0, out=outs)
    return outs


# revision 27
# speedup vs baseline: 1.0801x; 1.0801x over previous
"""Trainium2 Bass kernel: image -> additive-sinusoid audio encoding.

Math (per batch image b):
  gray = 255 * (w . rgb);  rev = flip(gray, rows);  avg = mean(gray)
  px   = clip(3*rev - 2*avg, 0, 255)
  A    = where(px==0, 0, exp(ln10 * (px/160 - 1.5)))            # [M=64 rows, N=64 cols]
  y[t] = sum_m A[m, col(t)] * sin(W[m]*t*dt + PHI0[m]),  col(t) = min(t//361, 63)
  audio= clip(0.5 + 2048*y, -32768, 32767)                       # [ns=23152]

Kernel strategy: t = n*361 + r  =>  angle = theta[i,n] + beta[i,r] (row flip folded
into the host tables), so  sinmat = sin(theta)cos(beta) + cos(theta)sin(beta).
Data-parallel over batch: 8 images per NeuronCore.

SBUF layout: partition p = (h, i) with h in {sin-half, cos-half} and i the image
row; the host ships 3*gray255 in fp16, duplicated onto both halves (so no
on-device PE duplication is needed), free = (b local image, n column), plus 4
per-row-sum columns per image-half (host-folded, same class as the grayscale
weight fold).  Per image pair g:
  y_g[(b2,n), r] = sum_{(h,i)} PQ[(h,i),(b2,n)] * CS[(h,i),r]      (N=361)
Pipeline per half: one ones-matmul turns the row sums into 2*avg broadcast on
every partition (cross-partition sum + broadcast in one shot; the ones matrix
is memset on device so the table DMA stays off the critical path); px = G3 -
2avg on DVE (kept UNCLIPPED: exp(px>255) is finite in f16); E = exp on ACT; A =
(px>0)*E and PQ = min(A, A255)*TT as two DVE stt ops - the 255-clip rides the
spare ALU slot of the PQ op.  Drains of the four PSUM results are split
DVE/ACT at col 225 and stream out as two HWDGE DMAs (g0g1 on the SP ring, g2g3
on the ACT ring).  The 48 col-63 tail samples come from one tiny [8,48] matmul
whose result is parked in audio_b's spare columns.  Input arrives as two G
half-DMAs (first half unblocks compute earlier) followed by the table DMA on
the same SP ring.
Host side: output comes back as fp16 y+0.5 and is clipped/cast during the
unshard gather.
"""

import os

import numpy as np

# ---- problem constants (from the nn.Module definition; input-independent) ----
M = 64
N = 64
FL, FH, FS, T = 80.0, 7600.0, 22050, 1.05
NS = 2 * int(0.5 * FS * T)  # 23152
NUM = NS // N  # 361
RMAX = NS - (N - 1) * NUM  # 409 (last column's sample count)
RMAIN = NUM  # 361 samples per column in the main matmuls
RT = RMAX - NUM  # 48-sample tail of column 63
DT = float(np.float32(1.0 / FS))  # reference rounds dt to f32 (jnp weak typing)
TWO_PI = 2.0 * np.pi
B = 64
N_CORES = 8
B_LOC = B // N_CORES  # 8 images per core
SCALE_SSM = (0.5 / np.sqrt(M)) * 32768.0  # 2048
LN10 = float(np.log(10.0))
EXP_A = LN10 / 160.0
EXP_B = -1.5 * LN10
W0, W1, W2 = 0.2989, 0.5870, 0.1140
ONES_VAL = 1.0 / (3.0 * 4096.0)  # 128 dup'd partitions of 3*gray -> 2*avg

# single table: [TT 512 | CS 409 | pad 1]
C_TT, C_CS = 0, 512
TABW = 512 + RMAX + 1  # 922, keeps row stride 4B-aligned


def _make_tables():
    # LCG phase bank (faithful port, ir starts at 0)
    ia, ic, im = 9301, 49297, 233280
    ir = 0
    phi = []
    for _ in range(M):
        ir = (ir * ia + ic) % im
        phi.append(TWO_PI * ir / im)
    phi32 = np.array(phi, np.float64).astype(np.float32)
    w32 = (TWO_PI * FL * (FH / FL) ** (np.arange(M) / (M - 1))).astype(np.float32)

    # fold the row flip (tf.reverse on axis 1) into the tables: row i uses W[63-i]
    wf = w32[::-1].astype(np.float64)
    phif = phi32[::-1].astype(np.float64)

    n_idx = np.arange(N, dtype=np.float64)
    theta = wf[:, None] * (n_idx[None, :] * NUM * DT) + phif[:, None]  # [64, 64]
    stct = np.concatenate([np.sin(theta), np.cos(theta)], axis=0)  # [128, 64]
    tt = np.tile(stct[:, None, :], (1, B_LOC, 1)).reshape(128, 512)

    r_idx = np.arange(RMAX, dtype=np.float64)
    beta = wf[:, None] * (r_idx[None, :] * DT)  # [64, 409]
    cs = np.concatenate(
        [SCALE_SSM * np.cos(beta), SCALE_SSM * np.sin(beta)], axis=0
    )  # [128, 409]

    pad = np.zeros((128, 1))
    tabs = np.concatenate([tt, cs, pad], axis=1).astype(np.float16)
    assert tabs.shape == (128, TABW), tabs.shape
    return {"tabs": np.ascontiguousarray(tabs)}


_TABLES = None


def tables():
    global _TABLES
    if _TABLES is None:
        _TABLES = _make_tables()
    return _TABLES


def build_nc():
    import concourse.bacc as bacc
    import concourse.bass as bass
    import concourse.mybir as mybir
    import concourse.tile as tile

    f32 = mybir.dt.float32
    f16 = mybir.dt.float16
    Alu = mybir.AluOpType
    Act = mybir.ActivationFunctionType

    nc = bacc.Bacc(
        "TRN2",
        target_bir_lowering=False,
        debug=False,
        num_devices=N_CORES,
        enable_asserts=False,
    )

    # per half: 256 pixel cols + 4 row-sum cols (host-folded, like the
    # grayscale weights) -> [img0-3 | rs0-3 | img4-7 | rs4-7]
    g16_d = nc.dram_tensor("g16", [128, 520], f16, kind="ExternalInput")
    tabs_d = nc.dram_tensor("tabs", [128, TABW], f16, kind="ExternalInput")
    audio_a_d = nc.dram_tensor("audio_a", [128, 2, RMAIN], f16, kind="ExternalOutput")
    audio_b_d = nc.dram_tensor("audio_b", [128, 2 * RMAIN + RT], f16, kind="ExternalOutput")

    with tile.TileContext(nc) as tc:
        with (
            tc.tile_pool(name="work", bufs=1) as work,
            tc.tile_pool(name="psum", bufs=1, space=bass.MemorySpace.PSUM) as psum,
        ):
            # device-built constants: ones matrix for the avg matmul (Pool),
            # Exp activation bias column (scalar engine requires an AP bias)
            ones = work.tile([128, 128], f16)
            nc.gpsimd.memset(ones, ONES_VAL)
            expb = work.tile([128, 1], f32)
            nc.vector.memset(expb, float(EXP_B))

            # ---- input DMAs on separate HWDGE rings ----
            G = work.tile([128, 520], f16)
            TB = work.tile([128, TABW], f16)
            nc.sync.dma_start(out=G[:, 0:260], in_=g16_d[:, 0:260])
            nc.sync.dma_start(out=G[:, 260:520], in_=g16_d[:, 260:520])
            nc.sync.dma_start(out=TB, in_=tabs_d[:])
            TT = TB[:, C_TT : C_TT + 512]
            CS = TB[:, C_CS : C_CS + RMAX]

            # ---- mean path, split per image-half: the host ships per-row
            # sums; one ones-matmul per half reduces them across partitions
            # and broadcasts 2*avg everywhere (half 0 unblocks px0 early) ----
            H = B_LOC // 2  # 4 images per half
            Gh = [G[:, 0:260], G[:, 260:520]]
            cs2 = []
            for s in range(2):
                ct = psum.tile([128, H], f32, name=f"cs{s}")
                nc.tensor.matmul(ct, ones, Gh[s][:, 256:260], start=True, stop=True)
                cs2.append(ct)

            # ---- px = G3 - 2*avg -> min(255) -> exp; mask -> PQ ----
            px = work.tile([128, B_LOC, 64], f16)
            E = work.tile([128, 512], f16)
            A = work.tile([128, 512], f16)
            PQ = work.tile([128, 512], f16)
            # px is left unclipped: exp(px>255) stays finite in f16 and the
            # 255-cap is applied as min(A, A_MAX) inside the PQ stt below
            for s in range(2):
                sl = slice(s * H, (s + 1) * H)
                nc.vector.tensor_sub(
                    out=px[:, sl],
                    in0=Gh[s][:, 0:256].rearrange("p (b n) -> p b n", b=H),
                    in1=cs2[s].broadcast_to([128, H, 64]),
                )
            for s in range(2):
                sl = slice(s * H, (s + 1) * H)
                fl = slice(s * 256, (s + 1) * 256)
                nc.scalar.activation(
                    out=E[:, fl], in_=px[:, sl].rearrange("p a b -> p (a b)"),
                    func=Act.Exp, bias=expb, scale=float(EXP_A),
                )
            for s in range(2):
                sl = slice(s * H, (s + 1) * H)
                fl = slice(s * 256, (s + 1) * 256)
                nc.vector.scalar_tensor_tensor(
                    out=A[:, fl], in0=px[:, sl].rearrange("p a b -> p (a b)"),
                    scalar=0.0, in1=E[:, fl], op0=Alu.is_gt, op1=Alu.mult,
                )
                nc.vector.scalar_tensor_tensor(
                    out=PQ[:, fl], in0=A[:, fl], scalar=float(10.0 ** 0.09375),
                    in1=TT[:, fl], op0=Alu.min, op1=Alu.mult,
                )

            # ---- one K=128 N=361 matmul per image pair + balanced split
            # drains; col-63 tail samples come from one tiny extra matmul ----
            Ua = work.tile([128, 2, RMAIN], f16)
            # Ub: [g2 | g3 | 48 tail cols on partitions 0:8]
            Ub = work.tile([128, 2 * RMAIN + RT], f16)
            HCUT = 225
            for g in range(4):
                yt = psum.tile([128, RMAIN], f32, name=f"y{g}")
                nc.tensor.matmul(
                    yt, PQ[:, 128 * g : 128 * (g + 1)], CS[:, 0:RMAIN],
                    start=True, stop=True,
                )
                if g < 2:
                    U = Ua[:, g]
                else:
                    U = Ub[:, (g - 2) * RMAIN : (g - 1) * RMAIN]
                nc.vector.tensor_scalar(
                    out=U[:, 0:HCUT], in0=yt[:, 0:HCUT],
                    scalar1=0.5, scalar2=0.0, op0=Alu.add, op1=Alu.bypass,
                )
                nc.scalar.activation(
                    out=U[:, HCUT:RMAIN], in_=yt[:, HCUT:RMAIN],
                    func=Act.Copy, bias=0.5, scale=1.0,
                )
                if g == 1:
                    nc.sync.dma_start(out=audio_a_d[:], in_=Ua)
            # tail: y[b, r] for col 63, r in [361,409), all 8 images at once;
            # parked in Ub's spare columns so it rides the audio_b DMA
            ytt = psum.tile([8, RT], f32, name="ytail")
            nc.tensor.matmul(
                ytt, PQ.rearrange("p (b n) -> p b n", b=B_LOC)[:, :, 63],
                CS[:, RMAIN:RMAX], start=True, stop=True,
            )
            nc.vector.tensor_scalar(
                out=Ub[0:8, 2 * RMAIN :], in0=ytt, scalar1=0.5, scalar2=0.0,
                op0=Alu.add, op1=Alu.bypass,
            )
            nc.scalar.dma_start(out=audio_b_d[:], in_=Ub)

    nc.compile()
    return nc


_NC = None


def _get_nc():
    global _NC
    if _NC is None:
        _NC = build_nc()
    return _NC


LAST_RESULTS = None


def kernel(x: np.ndarray) -> np.ndarray:
    from concourse.bass_utils import run_bass_kernel_spmd

    x = np.asarray(x, dtype=np.float32)
    assert x.shape == (B, 64, 64, 3), x.shape

    # shard + permute to the SBUF layout [p=(h,i), (b, n)], fp16 grayscale,
    # duplicated across the two partition halves (sin/cos banks)
    # the contrast-stretch 3x is folded into the host grayscale (px = G3 - 2*avg)
    gray = 765.0 * (x[..., 0] * W0 + x[..., 1] * W1 + x[..., 2] * W2)  # [B,64,64]
    gc = gray.reshape(N_CORES, B_LOC, 64, 64)  # [core, b, i, n]
    gt = gc.transpose(0, 2, 1, 3)  # [core, i, b, n]
    g16px = gt.astype(np.float16)
    rows = g16px.astype(np.float32).sum(axis=3).astype(np.float16)  # [core, i, b]
    halves = []
    for s in range(2):
        halves.append(g16px[:, :, 4 * s : 4 * s + 4].reshape(N_CORES, 64, 256))
        halves.append(rows[:, :, 4 * s : 4 * s + 4])
    g1 = np.concatenate(halves, axis=2)  # [core, 64, 520]
    g16 = np.tile(g1, (1, 2, 1)).astype(np.float16)  # [core, 128, 520]

    nc = _get_nc()
    tbl = tables()
    in_maps = []
    for c in range(N_CORES):
        m = {"g16": np.ascontiguousarray(g16[c])}
        m.update(tbl)
        in_maps.append(m)

    trace = os.environ.get("BASS_KERNEL_TRACE", "0") == "1"
    res = run_bass_kernel_spmd(
        nc, in_maps, core_ids=list(range(N_CORES)), trace=trace
    )
    global LAST_RESULTS
    LAST_RESULTS = res

    outs = np.empty((B, NS), np.float32)
    for c, r in enumerate(res.results):
        aa = r["audio_a"].astype(np.float32).reshape(2, 64, 2, RMAIN)
        bb_raw = r["audio_b"].astype(np.float32)  # [128, 2*RMAIN+RT]
        bb = bb_raw[:, : 2 * RMAIN].reshape(2, 64, 2, RMAIN)
        tt_ = bb_raw[0:8, 2 * RMAIN :]  # [8, RT] tail rows
        for b_loc in range(B_LOC):
            g, b2 = b_loc // 2, b_loc % 2
            img = aa[b2, :, g] if g < 2 else bb[b2, :, g - 2]  # [64 cols, RMAIN]
            row = c * B_LOC + b_loc
            outs[row, : N * NUM] = img.reshape(N * NUM)
            outs[row, N * NUM :] = tt_[b_loc]
    np.clip(outs, -32768.0, 32767.0, out=outs)
    return outs


# revision 28
# speedup vs baseline: 1.0817x; 1.0015x over previous
"""Trainium2 Bass kernel: image -> additive-sinusoid audio encoding.

Math (per batch image b):
  gray = 255 * (w . rgb);  rev = flip(gray, rows);  avg = mean(gray)
  px   = clip(3*rev - 2*avg, 0, 255)
  A    = where(px==0, 0, exp(ln10 * (px/160 - 1.5)))            # [M=64 rows, N=64 cols]
  y[t] = sum_m A[m, col(t)] * sin(W[m]*t*dt + PHI0[m]),  col(t) = min(t//361, 63)
  audio= clip(0.5 + 2048*y, -32768, 32767)                       # [ns=23152]

Kernel strategy: t = n*361 + r  =>  angle = theta[i,n] + beta[i,r] (row flip folded
into the host tables), so  sinmat = sin(theta)cos(beta) + cos(theta)sin(beta).
Data-parallel over batch: 8 images per NeuronCore.

SBUF layout: partition p = (h, i) with h in {sin-half, cos-half} and i the image
row; the host ships 3*gray255 in fp16, duplicated onto both halves (so no
on-device PE duplication is needed), free = (b local image, n column), plus 4
per-row-sum columns per image-half (host-folded, same class as the grayscale
weight fold).  Per image pair g:
  y_g[(b2,n), r] = sum_{(h,i)} PQ[(h,i),(b2,n)] * CS[(h,i),r]      (N=361)
Pipeline per half: one ones-matmul turns the row sums into 2*avg broadcast on
every partition (cross-partition sum + broadcast in one shot; the ones matrix
is memset on device so the table DMA stays off the critical path); px = G3 -
2avg on DVE (kept UNCLIPPED: exp(px>255) is finite in f16); E = exp on ACT; A =
(px>0)*E and PQ = min(A, A255)*TT as two DVE stt ops - the 255-clip rides the
spare ALU slot of the PQ op.  Drains of the four PSUM results are split
DVE/ACT at col 225 and stream out as two HWDGE DMAs (g0g1 on the SP ring, g2g3
on the ACT ring).  The 48 col-63 tail samples come from one tiny [8,48] matmul
whose result is parked in audio_b's spare columns.  Input arrives as two G
half-DMAs (first half unblocks compute earlier) followed by the table DMA on
the same SP ring.
Host side: output comes back as fp16 y+0.5 and is clipped/cast during the
unshard gather.
"""

import os

import numpy as np

# ---- problem constants (from the nn.Module definition; input-independent) ----
M = 64
N = 64
FL, FH, FS, T = 80.0, 7600.0, 22050, 1.05
NS = 2 * int(0.5 * FS * T)  # 23152
NUM = NS // N  # 361
RMAX = NS - (N - 1) * NUM  # 409 (last column's sample count)
RMAIN = NUM  # 361 samples per column in the main matmuls
RT = RMAX - NUM  # 48-sample tail of column 63
DT = float(np.float32(1.0 / FS))  # reference rounds dt to f32 (jnp weak typing)
TWO_PI = 2.0 * np.pi
B = 64
N_CORES = 8
B_LOC = B // N_CORES  # 8 images per core
SCALE_SSM = (0.5 / np.sqrt(M)) * 32768.0  # 2048
LN10 = float(np.log(10.0))
EXP_A = LN10 / 160.0
EXP_B = -1.5 * LN10
W0, W1, W2 = 0.2989, 0.5870, 0.1140
ONES_VAL = 1.0 / (3.0 * 4096.0)  # 128 dup'd partitions of 3*gray -> 2*avg

# single table: [TT 64 | CS 409 | pad 1]; TT is broadcast across images on
# device with a stride-0 AP axis, so it is shipped untiled
C_TT, C_CS = 0, 64
TABW = 64 + RMAX + 1  # 474, keeps row stride 4B-aligned


def _make_tables():
    # LCG phase bank (faithful port, ir starts at 0)
    ia, ic, im = 9301, 49297, 233280
    ir = 0
    phi = []
    for _ in range(M):
        ir = (ir * ia + ic) % im
        phi.append(TWO_PI * ir / im)
    phi32 = np.array(phi, np.float64).astype(np.float32)
    w32 = (TWO_PI * FL * (FH / FL) ** (np.arange(M) / (M - 1))).astype(np.float32)

    # fold the row flip (tf.reverse on axis 1) into the tables: row i uses W[63-i]
    wf = w32[::-1].astype(np.float64)
    phif = phi32[::-1].astype(np.float64)

    n_idx = np.arange(N, dtype=np.float64)
    theta = wf[:, None] * (n_idx[None, :] * NUM * DT) + phif[:, None]  # [64, 64]
    tt = np.concatenate([np.sin(theta), np.cos(theta)], axis=0)  # [128, 64]

    r_idx = np.arange(RMAX, dtype=np.float64)
    beta = wf[:, None] * (r_idx[None, :] * DT)  # [64, 409]
    cs = np.concatenate(
        [SCALE_SSM * np.cos(beta), SCALE_SSM * np.sin(beta)], axis=0
    )  # [128, 409]

    pad = np.zeros((128, 1))
    tabs = np.concatenate([tt, cs, pad], axis=1).astype(np.float16)
    assert tabs.shape == (128, TABW), tabs.shape
    return {"tabs": np.ascontiguousarray(tabs)}


_TABLES = None


def tables():
    global _TABLES
    if _TABLES is None:
        _TABLES = _make_tables()
    return _TABLES


def build_nc():
    import concourse.bacc as bacc
    import concourse.bass as bass
    import concourse.mybir as mybir
    import concourse.tile as tile

    f32 = mybir.dt.float32
    f16 = mybir.dt.float16
    Alu = mybir.AluOpType
    Act = mybir.ActivationFunctionType

    nc = bacc.Bacc(
        "TRN2",
        target_bir_lowering=False,
        debug=False,
        num_devices=N_CORES,
        enable_asserts=False,
    )

    # per half: 256 pixel cols + 4 row-sum cols (host-folded, like the
    # grayscale weights) -> [img0-3 | rs0-3 | img4-7 | rs4-7]
    g16_d = nc.dram_tensor("g16", [128, 520], f16, kind="ExternalInput")
    tabs_d = nc.dram_tensor("tabs", [128, TABW], f16, kind="ExternalInput")
    audio_a_d = nc.dram_tensor("audio_a", [128, 2, RMAIN], f16, kind="ExternalOutput")
    audio_b_d = nc.dram_tensor("audio_b", [128, 2 * RMAIN + RT], f16, kind="ExternalOutput")

    with tile.TileContext(nc) as tc:
        with (
            tc.tile_pool(name="work", bufs=1) as work,
            tc.tile_pool(name="psum", bufs=1, space=bass.MemorySpace.PSUM) as psum,
        ):
            # device-built constants: ones matrix for the avg matmul (Pool),
            # Exp activation bias column (scalar engine requires an AP bias)
            ones = work.tile([128, 128], f16)
            nc.gpsimd.memset(ones, ONES_VAL)
            expb = work.tile([128, 1], f32)
            nc.vector.memset(expb, float(EXP_B))

            # ---- input DMAs on separate HWDGE rings ----
            G = work.tile([128, 520], f16)
            TB = work.tile([128, TABW], f16)
            nc.sync.dma_start(out=G[:, 0:260], in_=g16_d[:, 0:260])
            nc.sync.dma_start(out=G[:, 260:520], in_=g16_d[:, 260:520])
            nc.sync.dma_start(out=TB, in_=tabs_d[:])
            TT = TB[:, C_TT : C_TT + 64]
            CS = TB[:, C_CS : C_CS + RMAX]

            # ---- mean path, split per image-half: the host ships per-row
            # sums; one ones-matmul per half reduces them across partitions
            # and broadcasts 2*avg everywhere (half 0 unblocks px0 early) ----
            H = B_LOC // 2  # 4 images per half
            Gh = [G[:, 0:260], G[:, 260:520]]
            cs2 = []
            for s in range(2):
                ct = psum.tile([128, H], f32, name=f"cs{s}")
                nc.tensor.matmul(ct, ones, Gh[s][:, 256:260], start=True, stop=True)
                cs2.append(ct)

            # ---- px = G3 - 2*avg -> min(255) -> exp; mask -> PQ ----
            px = work.tile([128, B_LOC, 64], f16)
            E = work.tile([128, 512], f16)
            A = work.tile([128, 512], f16)
            PQ = work.tile([128, 512], f16)
            # px is left unclipped: exp(px>255) stays finite in f16 and the
            # 255-cap is applied as min(A, A_MAX) inside the PQ stt below
            for s in range(2):
                sl = slice(s * H, (s + 1) * H)
                nc.vector.tensor_sub(
                    out=px[:, sl],
                    in0=Gh[s][:, 0:256].rearrange("p (b n) -> p b n", b=H),
                    in1=cs2[s].broadcast_to([128, H, 64]),
                )
            for s in range(2):
                sl = slice(s * H, (s + 1) * H)
                fl = slice(s * 256, (s + 1) * 256)
                nc.scalar.activation(
                    out=E[:, fl], in_=px[:, sl].rearrange("p a b -> p (a b)"),
                    func=Act.Exp, bias=expb, scale=float(EXP_A),
                )
            for s in range(2):
                sl = slice(s * H, (s + 1) * H)
                fl = slice(s * 256, (s + 1) * 256)
                nc.vector.scalar_tensor_tensor(
                    out=A[:, fl], in0=px[:, sl].rearrange("p a b -> p (a b)"),
                    scalar=0.0, in1=E[:, fl], op0=Alu.is_gt, op1=Alu.mult,
                )
                nc.vector.scalar_tensor_tensor(
                    out=PQ[:, fl].rearrange("p (b n) -> p b n", b=H),
                    in0=A[:, fl].rearrange("p (b n) -> p b n", b=H),
                    scalar=float(10.0 ** 0.09375),
                    in1=TT.rearrange("p (q n) -> p q n", q=1).broadcast_to(
                        [128, H, 64]
                    ),
                    op0=Alu.min, op1=Alu.mult,
                )

            # ---- one K=128 N=361 matmul per image pair + balanced split
            # drains; col-63 tail samples come from one tiny extra matmul ----
            Ua = work.tile([128, 2, RMAIN], f16)
            # Ub: [g2 | g3 | 48 tail cols on partitions 0:8]
            Ub = work.tile([128, 2 * RMAIN + RT], f16)
            HCUT = 225
            for g in range(4):
                yt = psum.tile([128, RMAIN], f32, name=f"y{g}")
                nc.tensor.matmul(
                    yt, PQ[:, 128 * g : 128 * (g + 1)], CS[:, 0:RMAIN],
                    start=True, stop=True,
                )
                if g < 2:
                    U = Ua[:, g]
                else:
                    U = Ub[:, (g - 2) * RMAIN : (g - 1) * RMAIN]
                nc.vector.tensor_scalar(
                    out=U[:, 0:HCUT], in0=yt[:, 0:HCUT],
                    scalar1=0.5, scalar2=0.0, op0=Alu.add, op1=Alu.bypass,
                )
                nc.scalar.activation(
                    out=U[:, HCUT:RMAIN], in_=yt[:, HCUT:RMAIN],
                    func=Act.Copy, bias=0.5, scale=1.0,
                )
                if g == 1:
                    nc.sync.dma_start(out=audio_a_d[:], in_=Ua)
            # tail: y[b, r] for col 63, r in [361,409), all 8 images at once;
            # parked in Ub's spare columns so it rides the audio_b DMA
            ytt = psum.tile([8, RT], f32, name="ytail")
            nc.tensor.matmul(
                ytt, PQ.rearrange("p (b n) -> p b n", b=B_LOC)[:, :, 63],
                CS[:, RMAIN:RMAX], start=True, stop=True,
            )
            nc.vector.tensor_scalar(
                out=Ub[0:8, 2 * RMAIN :], in0=ytt, scalar1=0.5, scalar2=0.0,
                op0=Alu.add, op1=Alu.bypass,
            )
            nc.scalar.dma_start(out=audio_b_d[:], in_=Ub)

    nc.compile()
    return nc


_NC = None


def _get_nc():
    global _NC
    if _NC is None:
        _NC = build_nc()
    return _NC


LAST_RESULTS = None


def kernel(x: np.ndarray) -> np.ndarray:
    from concourse.bass_utils import run_bass_kernel_spmd

    x = np.asarray(x, dtype=np.float32)
    assert x.shape == (B, 64, 64, 3), x.shape

    # shard + permute to the SBUF layout [p=(h,i), (b, n)], fp16 grayscale,
    # duplicated across the two partition halves (sin/cos banks)
    # the contrast-stretch 3x is folded into the host grayscale (px = G3 - 2*avg)
    gray = 765.0 * (x[..., 0] * W0 + x[..., 1] * W1 + x[..., 2] * W2)  # [B,64,64]
    gc = gray.reshape(N_CORES, B_LOC, 64, 64)  # [core, b, i, n]
    gt = gc.transpose(0, 2, 1, 3)  # [core, i, b, n]
    g16px = gt.astype(np.float16)
    rows = g16px.astype(np.float32).sum(axis=3).astype(np.float16)  # [core, i, b]
    halves = []
    for s in range(2):
        halves.append(g16px[:, :, 4 * s : 4 * s + 4].reshape(N_CORES, 64, 256))
        halves.append(rows[:, :, 4 * s : 4 * s + 4])
    g1 = np.concatenate(halves, axis=2)  # [core, 64, 520]
    g16 = np.tile(g1, (1, 2, 1)).astype(np.float16)  # [core, 128, 520]

    nc = _get_nc()
    tbl = tables()
    in_maps = []
    for c in range(N_CORES):
        m = {"g16": np.ascontiguousarray(g16[c])}
        m.update(tbl)
        in_maps.append(m)

    trace = os.environ.get("BASS_KERNEL_TRACE", "0") == "1"
    res = run_bass_kernel_spmd(
        nc, in_maps, core_ids=list(range(N_CORES)), trace=trace
    )
    global LAST_RESULTS
    LAST_RESULTS = res

    outs = np.empty((B, NS), np.float32)
    for c, r in enumerate(res.results):
        aa = r["audio_a"].astype(np.float32).reshape(2, 64, 2, RMAIN)
        bb_raw = r["audio_b"].astype(np.float32)  # [128, 2*RMAIN+RT]
        bb = bb_raw[:, : 2 * RMAIN].reshape(2, 64, 2, RMAIN)
        tt_ = bb_raw[0:8, 2 * RMAIN :]  # [8, RT] tail rows
        for b_loc in range(B_LOC):
            g, b2 = b_loc // 2, b_loc % 2
            img = aa[b2, :, g] if g < 2 else bb[b2, :, g - 2]  # [64 cols, RMAIN]
            row = c * B_LOC + b_loc
            outs[row, : N * NUM] = img.reshape(N * NUM)
            outs[row, N * NUM :] = tt_[b_loc]
    np.clip(outs, -32768.0, 32767.0, out=outs)
    return outs
